# revision 13
# baseline (speedup 1.0000x reference)
"""Trainium2 Bass kernel for nn_CrossAttentionLayer (ragged cross-attention + MLP).

Sharding: 64 ragged segments -> 8 cores x 8 segments, each segment padded to
LMAX=512 slots. All activations are kept channel-major ("transposed", [chan, tok])
so every matmul contracts over the partition dim with no on-device transposes
except V (PE-transpose). Softmax runs in scoresT orientation [m_src, n_dst]:
the src-padding mask is a per-partition bias on the exp; the denominator is
computed with ones-lhsT matmuls that replicate each head's denominator across
its 32 partitions (so the normalization is a plain reciprocal + multiply).
Merge conv + BN are folded into the MLP weights on the host.
"""
import math
import sys
from contextlib import ExitStack

import numpy as np

try:
    import concourse.bass as bass
except ImportError:
    sys.path.insert(0, "/opt/trn_rl_repo")
    import concourse.bass as bass

import concourse.tile as tile
from concourse import bacc, mybir
from concourse.bass_utils import run_bass_kernel_spmd

F32 = mybir.dt.float32
F32R = mybir.dt.float32r

B = 64
LMAX = 512
H = 256          # h_dim
C = 128          # h_div
HEADS = 4
DH = 32
NCORES = 8
SEGS = 8         # segments per core
T = SEGS * LMAX  # padded tokens per core
NPB = 7          # per-partition bias columns: bq,bk,bv,b1a,b1b,b2a,b2b
MASK_NEG = -20000.0


def _r(ap):
    return ap if ap.dtype == F32R else ap.bitcast(F32R)


def host_prep(inputs):
    src_h = np.ascontiguousarray(np.asarray(inputs['src_h'], np.float32))
    dst_h = np.ascontiguousarray(np.asarray(inputs['dst_h'], np.float32))
    ns = np.asarray(inputs['src_num_verts']).astype(np.int64)
    nd = np.asarray(inputs['dst_num_verts']).astype(np.int64)
    soff = np.concatenate([[0], np.cumsum(ns)[:-1]])
    doff = np.concatenate([[0], np.cumsum(nd)[:-1]])

    perm = np.empty(C, np.int64)
    for chat in range(C):
        h, d = divmod(chat, DH)
        perm[chat] = d * HEADS + h
    s = 1.0 / math.sqrt(DH)

    f32 = lambda k: np.asarray(inputs[k], np.float32)
    Wq, bq = f32('Wq'), f32('bq')
    Wk, bk = f32('Wk'), f32('bk')
    Wv, bv = f32('Wv'), f32('bv')
    Wm, bm = f32('Wm'), f32('bm')
    W1, b1 = f32('W1'), f32('b1')
    W2, b2 = f32('W2'), f32('b2')
    g1, be1, rm1, rv1 = f32('g1'), f32('be1'), f32('rm1'), f32('rv1')
    g2, be2, rm2, rv2 = f32('g2'), f32('be2'), f32('rm2'), f32('rv2')

    WqT = np.ascontiguousarray((Wq[perm] * s).T)          # [256,128]
    bq_s = bq[perm] * s
    WkT = np.ascontiguousarray(Wk[perm].T)
    bk_r = bk[perm]
    WvT = np.ascontiguousarray(Wv[perm].T)
    bv_r = bv[perm]
    Wm_p = Wm[:, perm]
    a1 = g1 / np.sqrt(rv1 + 1e-5)
    W1_f = W1 * a1[:, None]
    b1_f = b1 * a1 + be1 - rm1 * a1
    a2 = g2 / np.sqrt(rv2 + 1e-5)
    W2_f = W2 * a2[:, None]
    b2_f = b2 * a2 + be2 - rm2 * a2
    W1m_p = W1_f[:, H:] @ Wm_p
    b1_p = b1_f + W1_f[:, H:] @ bm
    W1T = np.ascontiguousarray(np.concatenate([W1_f[:, :H], W1m_p], axis=1).T)  # [384,256]
    W2T = np.ascontiguousarray(W2_f.T)                    # [256,256]

    pbias = np.zeros((128, NPB), np.float32)
    pbias[:, 0] = bq_s
    pbias[:, 1] = bk_r
    pbias[:, 2] = bv_r
    pbias[:, 3] = b1_p[:128]
    pbias[:, 4] = b1_p[128:]
    pbias[:, 5] = b2_f[:128]
    pbias[:, 6] = b2_f[128:]

    cores = []
    for c in range(NCORES):
        dstT = np.zeros((H, T), np.float32)
        srcT = np.zeros((H, T), np.float32)
        maskb = np.full((128, SEGS * 4), MASK_NEG, np.float32)
        for si in range(SEGS):
            g = c * SEGS + si
            dstT[:, si * LMAX: si * LMAX + nd[g]] = dst_h[doff[g]:doff[g] + nd[g]].T
            srcT[:, si * LMAX: si * LMAX + ns[g]] = src_h[soff[g]:soff[g] + ns[g]].T
            for j in range(4):
                valid = max(0, min(128, int(ns[g]) - j * 128))
                maskb[:valid, si * 4 + j] = 0.0
        cores.append(dict(dstT=dstT, srcT=srcT, maskb=maskb))

    onespad = np.zeros((128, HEADS, C), np.float32)
    for h in range(HEADS):
        onespad[:, h, h * DH:(h + 1) * DH] = 1.0
    shared = dict(WqT=WqT, WkT=WkT, WvT=WvT, W1T=W1T, W2T=W2T, pbias=pbias,
                  eye=np.eye(128, dtype=np.float32),
                  onespad=onespad)
    meta = dict(nd=nd, doff=doff)
    return cores, shared, meta


def declare_tensors(nc):
    aps = {}
    aps['dstT'] = nc.dram_tensor("dstT", [H, T], F32R, kind="ExternalInput").ap()
    aps['srcT'] = nc.dram_tensor("srcT", [H, T], F32R, kind="ExternalInput").ap()
    aps['maskb'] = nc.dram_tensor("maskb", [128, SEGS * 4], F32, kind="ExternalInput").ap()
    aps['WqT'] = nc.dram_tensor("WqT", [H, C], F32R, kind="ExternalInput").ap()
    aps['WkT'] = nc.dram_tensor("WkT", [H, C], F32R, kind="ExternalInput").ap()
    aps['WvT'] = nc.dram_tensor("WvT", [H, C], F32R, kind="ExternalInput").ap()
    aps['W1T'] = nc.dram_tensor("W1T", [H + C, H], F32R, kind="ExternalInput").ap()
    aps['W2T'] = nc.dram_tensor("W2T", [H, H], F32R, kind="ExternalInput").ap()
    aps['pbias'] = nc.dram_tensor("pbias", [128, NPB], F32, kind="ExternalInput").ap()
    aps['eye'] = nc.dram_tensor("eye", [128, 128], F32R, kind="ExternalInput").ap()
    aps['onespad'] = nc.dram_tensor("onespad", [128, HEADS * C], F32R, kind="ExternalInput").ap()
    aps['vzero'] = nc.dram_tensor("vzero", [128, 4 * HEADS * C], F32R, kind="ExternalInput").ap()
    aps['outT'] = nc.dram_tensor("outT", [H, T], F32, kind="ExternalOutput").ap()
    return aps


def build_body(ctx: ExitStack, tc: tile.TileContext, aps):
    nc = tc.nc
    dstT_d, srcT_d, outT_d = aps['dstT'], aps['srcT'], aps['outT']

    wp = ctx.enter_context(tc.tile_pool(name="wp", bufs=1))
    inp = ctx.enter_context(tc.tile_pool(name="inp", bufs=1))
    qkv = ctx.enter_context(tc.tile_pool(name="qkv", bufs=1))
    att = ctx.enter_context(tc.tile_pool(name="att", bufs=1))
    mls = ctx.enter_context(tc.tile_pool(name="mls", bufs=1))
    # PSUM pools: gp (proj+mlp, 2 banks) + sc (scores, 2x2 banks) + md (msg+den, 2 banks)
    gp = ctx.enter_context(tc.tile_pool(name="gp", bufs=2, space="PSUM"))
    scp = ctx.enter_context(tc.tile_pool(name="scp", bufs=2, space="PSUM"))
    mdp = ctx.enter_context(tc.tile_pool(name="mdp", bufs=2, space="PSUM"))

    # --- weights ---
    wq = wp.tile([128, 2, C], F32R, tag="wq")
    wk = wp.tile([128, 2, C], F32R, tag="wk")
    wv = wp.tile([128, 2, C], F32R, tag="wv")
    w1 = wp.tile([128, 3, H], F32R, tag="w1")
    w2 = wp.tile([128, 2, H], F32R, tag="w2")
    pb = wp.tile([128, NPB], F32, tag="pb")
    maskb_t = wp.tile([128, SEGS * 4], F32, tag="maskb")
    onespad = wp.tile([128, HEADS, C], F32R, tag="onespad")
    eye = wp.tile([128, 128], F32R, tag="eye")
    for a in range(2):
        nc.sync.dma_start(out=wq[:, a, :], in_=aps['WqT'][a * 128:(a + 1) * 128, :])
        nc.sync.dma_start(out=wk[:, a, :], in_=aps['WkT'][a * 128:(a + 1) * 128, :])
        nc.sync.dma_start(out=wv[:, a, :], in_=aps['WvT'][a * 128:(a + 1) * 128, :])
        nc.sync.dma_start(out=w2[:, a, :], in_=aps['W2T'][a * 128:(a + 1) * 128, :])
    for a in range(3):
        nc.sync.dma_start(out=w1[:, a, :], in_=aps['W1T'][a * 128:(a + 1) * 128, :])
    nc.sync.dma_start(out=pb[:], in_=aps['pbias'][:])
    nc.sync.dma_start(out=maskb_t[:], in_=aps['maskb'][:])
    nc.sync.dma_start(out=onespad[:], in_=aps['onespad'].rearrange("p (h c) -> p h c", h=HEADS))
    nc.sync.dma_start(out=eye[:], in_=aps['eye'][:])

    # --- persistent V slots (zero-padded band layout), zero-filled once ---
    v_slots = []
    for i in range(3):
        vs = qkv.tile([128, 4, HEADS, C], F32R, tag=f"Vs{i}", name=f"Vs{i}")
        nc.sync.dma_start(out=vs[:], in_=aps['vzero'].rearrange("p (a h c) -> p a h c", a=4, h=HEADS))
        v_slots.append(vs)

    # --- persistent input tiles ---
    dst_t = [[None] * SEGS for _ in range(2)]
    src_t = [[None] * SEGS for _ in range(2)]
    for a in range(2):
        for s in range(SEGS):
            dt_ = inp.tile([128, LMAX], F32R, tag=f"dst{a}_{s}", name=f"dst{a}_{s}")
            nc.sync.dma_start(out=dt_[:], in_=dstT_d[a * 128:(a + 1) * 128, s * LMAX:(s + 1) * LMAX])
            dst_t[a][s] = dt_
            st_ = inp.tile([128, LMAX], F32R, tag=f"src{a}_{s}", name=f"src{a}_{s}")
            nc.sync.dma_start(out=st_[:], in_=srcT_d[a * 128:(a + 1) * 128, s * LMAX:(s + 1) * LMAX])
            src_t[a][s] = st_

    for s in range(SEGS):
        # ---------- projections for segment s ----------
        ps_q = gp.tile([128, LMAX], F32, tag="gp", name=f"psq{s}")
        for a in range(2):
            nc.tensor.matmul(ps_q[:], _r(wq[:, a, :]), _r(dst_t[a][s][:]),
                             start=(a == 0), stop=(a == 1))
        q_t = qkv.tile([128, LMAX], F32R, tag=f"q{s}", name=f"q{s}")
        nc.vector.tensor_scalar_add(q_t[:], ps_q[:], pb[:, 0:1])

        ps_k = gp.tile([128, LMAX], F32, tag="gp", name=f"psk{s}")
        for a in range(2):
            nc.tensor.matmul(ps_k[:], _r(wk[:, a, :]), _r(src_t[a][s][:]),
                             start=(a == 0), stop=(a == 1))
        k_t = qkv.tile([128, LMAX], F32R, tag=f"k{s}", name=f"k{s}")
        nc.vector.tensor_scalar_add(k_t[:], ps_k[:], pb[:, 1:2])

        ps_v = gp.tile([128, LMAX], F32, tag="gp", name=f"psv{s}")
        for a in range(2):
            nc.tensor.matmul(ps_v[:], _r(wv[:, a, :]), _r(src_t[a][s][:]),
                             start=(a == 0), stop=(a == 1))
        vT_t = qkv.tile([128, LMAX], F32R, tag="vt", name=f"vt{s}", bufs=2)
        nc.vector.tensor_scalar_add(vT_t[:], ps_v[:], pb[:, 2:3])

        # transpose vT [chan,tok] -> V natural [tok, chan] (4 chunks of 128 toks)
        ps_tr = gp.tile([128, 4, 128], F32R, tag="gp", name=f"pstr{s}")
        for j in range(4):
            nc.tensor.transpose(ps_tr[:, j, :], vT_t[:, j * 128:(j + 1) * 128], eye[:])
        v_sb = v_slots[s % 3]
        # scatter each transposed [128,128] j-block into its 4 per-head bands
        vdst = bass.AP(tensor=v_sb.tensor, offset=v_sb.offset,
                       ap=[v_sb.ap[0]] + [[HEADS * C, 4], [C + DH, HEADS], [1, DH]])
        vsrc = bass.AP(tensor=ps_tr.tensor, offset=ps_tr.offset,
                       ap=[ps_tr.ap[0]] + [[128, 4], [DH, HEADS], [1, DH]])
        nc.vector.tensor_copy(vdst, vsrc)

        # ---------- attention for segment s ----------
        ps_msg = mdp.tile([128, LMAX], F32, tag="md", name=f"psmsg{s}")
        ps_den = mdp.tile([128, LMAX], F32, tag="md", name=f"psden{s}")
        for j in range(4):
            for hp in range(2):
                ps_sc = scp.tile([128, 2, LMAX], F32, tag="sc", name=f"pssc{s}_{j}_{hp}")
                for hh in range(2):
                    h = 2 * hp + hh
                    nc.tensor.matmul(
                        ps_sc[:, hh, :],
                        _r(k_t[32 * h:32 * h + 32, j * 128:(j + 1) * 128]),
                        _r(q_t[32 * h:32 * h + 32, :]),
                        start=True, stop=True, tile_position=(32 * h, 0))
                e_t = att.tile([128, 2, LMAX], F32R, tag="E", name=f"E{s}_{j}_{hp}", bufs=5)
                nc.scalar.activation(e_t[:], ps_sc[:],
                                     mybir.ActivationFunctionType.Exp,
                                     bias=maskb_t[:, s * 4 + j: s * 4 + j + 1])
                for hh in range(2):
                    h = 2 * hp + hh
                    first = (j == 0 and h == 0)
                    last = (j == 3 and h == 3)
                    nc.tensor.matmul(
                        ps_msg[:, :],
                        v_sb[:, j, h, :],
                        e_t[:, hh, :],
                        start=first, stop=last)
                    # den replicated over the head's 32 partitions: this IS the
                    # broadcast needed for the normalization divide below
                    nc.tensor.matmul(
                        ps_den[:, :],
                        onespad[:, h, :],
                        e_t[:, hh, :],
                        start=first, stop=last)
        r_sb = att.tile([128, LMAX], F32, tag="rsb", name=f"rsb{s}", bufs=2)
        nc.vector.reciprocal(r_sb[:], ps_den[:])
        msgn = att.tile([128, LMAX], F32R, tag="msgn", name=f"msgn{s}", bufs=3)
        nc.vector.tensor_mul(msgn[:], ps_msg[:], r_sb[:])

        # ---------- MLP for segment s (merge folded into W1) ----------
        y1 = [None, None]
        for o in range(2):
            ps_y = gp.tile([128, LMAX], F32, tag="gp", name=f"psy{s}_{o}")
            rhs_list = [dst_t[0][s], dst_t[1][s], msgn]
            for kk in range(3):
                nc.tensor.matmul(ps_y[:], _r(w1[:, kk, o * 128:(o + 1) * 128]),
                                 _r(rhs_list[kk][:]), start=(kk == 0), stop=(kk == 2))
            y1_t = mls.tile([128, LMAX], F32R, tag=f"y1_{o}", name=f"y1_{s}_{o}", bufs=2)
            nc.vector.tensor_scalar(y1_t[:], ps_y[:], pb[:, 3 + o:4 + o], 0.0,
                                    op0=mybir.AluOpType.add, op1=mybir.AluOpType.max)
            y1[o] = y1_t
        for o in range(2):
            ps_z = gp.tile([128, LMAX], F32, tag="gp", name=f"psz{s}_{o}")
            for kk in range(2):
                nc.tensor.matmul(ps_z[:], _r(w2[:, kk, o * 128:(o + 1) * 128]),
                                 _r(y1[kk][:]), start=(kk == 0), stop=False)
            nc.tensor.matmul(ps_z[:], _r(eye[:]), _r(dst_t[o][s][:]),
                             start=False, stop=True)
            out_sb = mls.tile([128, LMAX], F32, tag=f"out_{o}", name=f"out_{s}_{o}", bufs=2)
            nc.vector.tensor_scalar_add(out_sb[:], ps_z[:], pb[:, 5 + o:6 + o])
            nc.sync.dma_start(out=outT_d[o * 128:(o + 1) * 128, s * LMAX:(s + 1) * LMAX],
                              in_=out_sb[:])


def build_nc():
    nc = bacc.Bacc("TRN2", target_bir_lowering=False, debug=False,
                   enable_asserts=True, num_devices=NCORES)
    aps = declare_tensors(nc)
    with tile.TileContext(nc) as tc:
        with ExitStack() as ctx:
            build_body(ctx, tc, aps)
    nc.compile()
    return nc


def in_map(core, shared):
    m = dict(dstT=core['dstT'], srcT=core['srcT'], maskb=core['maskb'])
    m['onespad'] = shared['onespad'].reshape(128, HEADS * C)
    m['vzero'] = np.zeros((128, 4 * HEADS * C), np.float32)
    m.update({k: shared[k] for k in ('WqT', 'WkT', 'WvT', 'W1T', 'W2T', 'pbias', 'eye')})
    return m


def assemble(outTs, meta):
    nd = meta['nd']
    doff = meta['doff']
    out = np.empty((int(nd.sum()), H), np.float32)
    for c in range(NCORES):
        for si in range(SEGS):
            g = c * SEGS + si
            out[doff[g]:doff[g] + nd[g]] = outTs[c][:, si * LMAX: si * LMAX + nd[g]].T
    return out


def kernel(**inputs):
    cores, shared, meta = host_prep(inputs)
    nc = build_nc()
    in_maps = [in_map(cores[c], shared) for c in range(NCORES)]
    res = run_bass_kernel_spmd(nc, in_maps, core_ids=list(range(NCORES)))
    outTs = [res.results[c]["outT"] for c in range(NCORES)]
    return assemble(outTs, meta)
